# revision 42
# baseline (speedup 1.0000x reference)
"""GATConv forward on 8 Trainium2 NeuronCores (Bass/Tile).

Strategy: destination-node sharding. Host sorts edges by dst, assigns each
core a contiguous dst range (12544 nodes = 98 tiles of 128). Node ids are
cyclically renumbered per core so every core's local nodes are 0..12543 and
the SPMD program is identical across cores; all per-core variation lives in
the input data.

Phase A: per 128-node tile, [h | a_src | 0.5*a_dst] = x @ waug on the PE;
rows packed into a 512B-row HBM table T as [h bf16 (256B) | a_src fp16
(8B) | pad]; 0.5*a_dst for local tiles kept in SBUF (slot-partitioned,
fp16).

Phase B: dst tiles are bin-packed into groups of 3 and processed in a
software pipeline so each engine's in-order queue never head-of-line
blocks on another engine:
  iteration i issues  gather(i)+streams(i+1) | bitexpand(i) | a_dst(i)
  [PE] | z/w/msg(i-1) [DVE+Act] | scatter(i-1) [PE] | epilogue(i-2).
dma_gathers are capped at 8 chunks (1024-descriptor SWDGE ring limit).
One-hot matrices come from host-packed uint16 bitmaps bit-expanded with
(x & (1<<b)) << (14-b), whose u16 result 0x4000 bitcast to fp16 is exactly
2.0 (scale cancels in the softmax ratio; att_dst is pre-halved). a_dst per
edge = tiny PE matmuls s01T^T @ adst; w = exp(lrelu(a_src + a_dst));
messages [w*h | w] scatter into a per-group [128,3,132] PSUM accumulator;
epilogue relu(mean_h num/den + bias).
"""
import sys

sys.path.insert(0, "/opt/trn_rl_repo")
import numpy as np
import ml_dtypes

import concourse.bass as bass
import concourse.mybir as mybir
import concourse.tile as tile
from concourse.bass_utils import run_bass_kernel_spmd
from concourse import bacc

BF16 = ml_dtypes.bfloat16
P = 128
N = 100000
NPAD = 100352          # 784 tiles of 128; 8 cores x 12544
NCORE = 8
B = NPAD // NCORE      # 12544 local nodes per core
TPC = B // P           # 98 tiles per core
NTILE = NPAD // P      # 784 global row tiles
BANK = 32768
NBANK = (NPAD + BANK - 1) // BANK  # 4
NEG = 0.2
H, C = 4, 32
G = 3                  # dst tiles per group (bin-packed)
EW = 256               # T row width in u16 elements (512B)


def _prep_edges(edge_index):
    src0 = edge_index[0].astype(np.int64)
    dst0 = edge_index[1].astype(np.int64)
    loops = np.arange(NPAD, dtype=np.int64)
    src = np.concatenate([src0, loops])
    dst = np.concatenate([dst0, loops])

    per_core = []
    cnts = np.zeros((NCORE, TPC * NBANK), np.int64)
    for c in range(NCORE):
        lo, hi = c * B, (c + 1) * B
        sel = (dst >= lo) & (dst < hi)
        d = dst[sel] - lo
        s = (src[sel] - lo) % NPAD
        t = d >> 7
        sl = d & 127
        bk = s >> 15
        il = s & (BANK - 1)
        q = t * NBANK + bk
        per_core.append((q, il, sl))
        cnts[c] = np.bincount(q, minlength=TPC * NBANK)

    K = np.ceil(cnts.max(axis=0) / P).astype(np.int64).reshape(TPC, NBANK)

    # bin-pack tiles into groups of G, balancing total chunks per group
    w = K.sum(axis=1)
    order = np.argsort(-w, kind="stable")
    ngrp = (TPC + G - 1) // G
    bins = [[] for _ in range(ngrp)]
    load = np.zeros(ngrp, np.int64)
    for t in order:
        cand = [b for b in range(ngrp) if len(bins[b]) < G]
        b = min(cand, key=lambda i: load[i])
        bins[b].append(int(t))
        load[b] += w[t]
    groups = [sorted(b) for b in bins]

    # global chunk layout: (group, bank, tile, chunk)
    qorder = []
    for tiles in groups:
        for bk in range(NBANK):
            for t in tiles:
                qorder.append(t * NBANK + bk)
    qorder = np.array(qorder, np.int64)
    sz_by_q = (K.reshape(-1) * P)
    sz_in_order = sz_by_q[qorder]
    goff_in_order = np.zeros(len(qorder) + 1, np.int64)
    np.cumsum(sz_in_order, out=goff_in_order[1:])
    tot = int(goff_in_order[-1])
    qoff = np.zeros(TPC * NBANK, np.int64)
    qoff[qorder] = goff_in_order[:-1]

    TOTC = tot // P
    idx_maps, bm_maps = [], []
    for c in range(NCORE):
        q, il, sl = per_core[c]
        cnt = cnts[c]
        start = np.zeros(TPC * NBANK + 1, np.int64)
        np.cumsum(cnt, out=start[1:])
        order_e = np.argsort(q, kind="stable")
        qs = q[order_e]
        rank = np.arange(len(qs)) - start[qs]
        pos = qoff[qs] + rank               # global padded edge position
        idx_pad = np.zeros(tot, np.int16)
        idx_pad[pos] = il[order_e].astype(np.int16)

        # idx table: per chunk [16, 8] wrap -> [16, tot/16], replicated to 128
        idx16 = np.ascontiguousarray(
            idx_pad.reshape(TOTC, 8, 16).transpose(2, 0, 1).reshape(16, TOTC * 8))
        idx128 = np.ascontiguousarray(np.tile(idx16, (8, 1)))

        cc = pos >> 7                       # chunk of each real edge
        lane = pos & 127                    # partition lane within chunk
        slv = sl[order_e]                   # slot (dst & 127) of each edge
        # bmg[e, cc*8 + slot//16] bit slot%16  (edge-partitioned, bits=slot)
        bmg = np.zeros((P, TOTC * 8), np.uint16)
        np.bitwise_or.at(bmg, (lane, cc * 8 + (slv >> 4)),
                         (1 << (slv & 15)).astype(np.uint16))
        # bmt[slot, cc*8 + lane//16] bit lane%16 (slot-partitioned, bits=edge)
        bmt = np.zeros((P, TOTC * 8), np.uint16)
        np.bitwise_or.at(bmt, (slv, cc * 8 + (lane >> 4)),
                         (1 << (lane & 15)).astype(np.uint16))
        idx_maps.append(idx128)
        bm_maps.append((bmg, bmt))
    return K, groups, idx_maps, bm_maps


def _plan(K, groups):
    """Per-group program metadata (shared across cores)."""
    plan = []
    cc = 0
    for tiles in groups:
        gathers = []   # (bank, nch, chunk_off_in_group); nch <= 8
        off = 0
        for bk in range(NBANK):
            nch_bk = int(K[tiles, bk].sum())
            for p0 in range(0, nch_bk, 8):
                gathers.append((bk, min(8, nch_bk - p0), off + p0))
            off += nch_bk
        kg = off
        # chunk -> owning tile (local index), in (bank, tile, chunk) order
        owner = []
        for bk in range(NBANK):
            for ti, t in enumerate(tiles):
                owner += [ti] * int(K[t, bk])
        # scatter order: tile-major so accumulation groups don't interleave
        scatter = []   # (t_local, chunk idx, start, stop)
        for ti, t in enumerate(tiles):
            ks = [k for k in range(kg) if owner[k] == ti]
            for i, k in enumerate(ks):
                scatter.append((ti, k, i == 0, i == len(ks) - 1))
        plan.append(dict(tiles=tiles, gathers=gathers, kg=kg, cc0=cc,
                         owner=owner, scatter=scatter))
        cc += kg
    return plan, cc


def _interleave_bitmaps(plan, TOTC, bm_maps):
    out = []
    for bmg, bmt in bm_maps:
        m = np.zeros((P, TOTC * 16), np.uint16)
        for pl in plan:
            c0, kg = pl["cc0"], pl["kg"]
            m[:, c0 * 16:c0 * 16 + kg * 8] = bmg[:, c0 * 8:(c0 + kg) * 8]
            m[:, c0 * 16 + kg * 8:(c0 + kg) * 16] = bmt[:, c0 * 8:(c0 + kg) * 8]
        out.append(m)
    return out


def _build_program(K, groups):
    plan, TOTC = _plan(K, groups)
    NG = len(plan)
    TOT16 = TOTC * 8
    f32, bf16, fp16, fp8, i16, u16 = (
        mybir.dt.float32, mybir.dt.bfloat16, mybir.dt.float16,
        mybir.dt.float8e4, mybir.dt.int16, mybir.dt.uint16)
    AF = mybir.ActivationFunctionType
    OP = mybir.AluOpType

    nc = bacc.Bacc("TRN2", target_bir_lowering=False, debug=False,
                   num_devices=NCORE)
    xbf = nc.dram_tensor("xbf", [NPAD, P], bf16, kind="ExternalInput")
    waug = nc.dram_tensor("waug", [P, 136], bf16, kind="ExternalInput")
    idx_all = nc.dram_tensor("idx_all", [P, TOT16], i16, kind="ExternalInput")
    bm_all = nc.dram_tensor("bm_all", [P, TOTC * 16], u16, kind="ExternalInput")
    bias_in = nc.dram_tensor("bias_in", [P, C], f32, kind="ExternalInput")
    T = nc.dram_tensor("T", [NPAD, EW], u16)
    out_d = nc.dram_tensor("out", [B, C], f32, kind="ExternalOutput")

    Tv = T[:, :].rearrange("(t p) e -> p t e", p=P)         # [128, 784, 128]
    out_v = out_d[:, :].rearrange("(t p) c -> p t c", p=P)  # [128, 98, 32]

    with tile.TileContext(nc) as tc:
        with tc.tile_pool(name="const", bufs=1) as cp:
            waug_sb = cp.tile([P, 136], bf16)
            nc.sync.dma_start(out=waug_sb[:], in_=waug[:, :])
            bias_sb = cp.tile([P, C], f32)
            nc.sync.dma_start(out=bias_sb[:], in_=bias_in[:, :])
            adst_sb = cp.tile([P, TPC, 4], fp16)
            outall_sb = cp.tile([P, TPC, C], f32)

            # ------------- Phase A: T = x @ waug, a_dst table --------------
            # Chunked into few, large DMAs: the tile scheduler serializes
            # any DMA against the next one with a ~5.5us bubble, so one
            # 8192-row transpose-in and one 64-tile T-write-out per chunk
            # bounds that cost to ~13 bubbles total. T writes go through
            # HWDGE (Act queue) -- SWDGE's 1024-descriptor ring cannot take
            # an 8192-descriptor DMA.
            CH = 16384
            NCHUNK = (NPAD + CH - 1) // CH  # 7 (last chunk 2048 rows)
            with tc.tile_pool(name="pax", bufs=2) as pax, \
                 tc.tile_pool(name="pat", bufs=2) as pat, \
                 tc.tile_pool(name="psa", bufs=2, space="PSUM") as psa:
                for ck in range(NCHUNK):
                    r0 = ck * CH
                    nrows = min(CH, NPAD - r0)
                    nt = nrows // P                  # tiles in chunk (<=64)
                    xT = pax.tile([P, CH], bf16, tag="xT")
                    nc.sync.dma_start(out=xT[:, 0:nrows],
                                      in_=xbf[r0:r0 + nrows, :],
                                      transpose=True)
                    Tb = pat.tile([P, CH // P, 132], u16, tag="Tb")
                    for q in range(nt // 4):
                        ps = psa.tile([P, 4, 512], f32, tag="psA",
                                      space="PSUM")
                        for i in range(4):
                            nc.tensor.matmul(
                                out=ps[:, i, 0:136],
                                lhsT=xT[:, (q * 4 + i) * P:(q * 4 + i + 1) * P],
                                rhs=waug_sb[:], start=True, stop=True)
                        q4 = slice(q * 4, (q + 1) * 4)
                        if q % 2 == 0:
                            nc.scalar.activation(
                                out=Tb[:, q4, 0:128].bitcast(bf16),
                                in_=ps[:, :, 0:128], func=AF.Copy)
                        else:
                            nc.vector.tensor_copy(
                                out=Tb[:, q4, 0:128].bitcast(bf16),
                                in_=ps[:, :, 0:128])
                        nc.scalar.activation(
                            out=Tb[:, q4, 128:132].bitcast(fp16),
                            in_=ps[:, :, 128:132], func=AF.Copy)
                        t0 = r0 // P + q * 4
                        if t0 < TPC:
                            nloc = min(4, TPC - t0)
                            nc.scalar.activation(
                                out=adst_sb[:, t0:t0 + nloc, :],
                                in_=ps[:, 0:nloc, 132:136], func=AF.Copy)
                    nc.scalar.dma_start(
                        out=Tv[:, r0 // P:r0 // P + nt, 0:132],
                        in_=Tb[:, 0:nt, :])

            tc.strict_bb_all_engine_barrier()

            # ------------- Phase B: software-pipelined groups --------------
            with tc.tile_pool(name="pgt", bufs=2) as pgt, \
                 tc.tile_pool(name="pix", bufs=3) as pix, \
                 tc.tile_pool(name="pbm", bufs=3) as pbm, \
                 tc.tile_pool(name="ptr", bufs=4) as ptr, \
                 tc.tile_pool(name="pt1", bufs=1) as pt1, \
                 tc.tile_pool(name="psm", bufs=2) as psm, \
                 tc.tile_pool(name="pms", bufs=1) as pms, \
                 tc.tile_pool(name="psb", bufs=4, space="PSUM") as psb, \
                 tc.tile_pool(name="psd", bufs=3, space="PSUM") as psd:
                st = {}  # live per-group tiles
                nregs = {}
                for pl_ in plan:
                    for _, nch_, _ in pl_["gathers"]:
                        nregs.setdefault(nch_ * P, None)
                for v in sorted(nregs):
                    nregs[v] = nc.gpsimd.to_reg(v)

                def issue_streams(g):
                    pl = plan[g]
                    kg, cc0 = pl["kg"], pl["cc0"]
                    idx_t = pix.tile([P, kg * 8], i16, tag="idx")
                    nc.scalar.dma_start(
                        out=idx_t[:], in_=idx_all[:, cc0 * 8:(cc0 + kg) * 8])
                    bm_t = pbm.tile([P, 2, kg * 8], u16, tag="bm")
                    nc.scalar.dma_start(
                        out=bm_t[:].rearrange("p a b -> p (a b)"),
                        in_=bm_all[:, cc0 * 16:(cc0 + kg) * 16])
                    st[g] = {"idx": idx_t, "bm": bm_t}

                def issue_gather(g):
                    pl = plan[g]
                    kg = pl["kg"]
                    gt = pgt.tile([P, kg, EW], u16, tag="gath")
                    idx_t = st[g]["idx"]
                    for bk, nch, off in pl["gathers"]:
                        rows = min(BANK, NPAD - bk * BANK)
                        nc.gpsimd.dma_gather(
                            out_ap=gt[:, off:off + nch, :],
                            in_ap=T[bk * BANK:bk * BANK + rows, :],
                            idxs_ap=idx_t[:, off * 8:(off + nch) * 8],
                            num_idxs=nch * P, num_idxs_reg=nregs[nch * P],
                            elem_size=EW)
                    st[g]["gt"] = gt

                def issue_bitexp(g):
                    pl = plan[g]
                    kg = pl["kg"]
                    k8 = kg * 8
                    bm_t = st[g]["bm"]
                    bxg = ptr.tile([P, k8, 16], u16, tag="bxg")
                    bxt = pt1.tile([P, k8, 16], u16, tag="bxt")
                    for half, tile_, src_ in ((0, bxt, 1), (1, bxg, 0)):
                        for b in range(16):
                            sh = (OP.logical_shift_left if b <= 14
                                  else OP.logical_shift_right)
                            nc.vector.tensor_scalar(
                                out=tile_[:, :, b],
                                in0=bm_t[:, src_], scalar1=1 << b,
                                scalar2=abs(14 - b),
                                op0=OP.bitwise_and, op1=sh)
                    st[g]["s01g"] = bxg[:].bitcast(fp16).rearrange(
                        "p (k w) b -> p k (w b)", w=8)
                    st[g]["s01t"] = bxt[:].bitcast(fp16).rearrange(
                        "p (k w) b -> p k (w b)", w=8)

                def issue_adt(g):
                    pl = plan[g]
                    kg, tiles = pl["kg"], pl["tiles"]
                    s01t = st[g]["s01t"]
                    adt_ps = psd.tile([P, kg, 4], f32, tag="adt", space="PSUM")
                    for k in range(kg):
                        nc.tensor.matmul(
                            out=adt_ps[:, k, :], lhsT=s01t[:, k, :],
                            rhs=adst_sb[:, tiles[pl["owner"][k]], :],
                            start=True, stop=True)
                    st[g]["adt"] = adt_ps

                def issue_z(g):
                    pl = plan[g]
                    kg = pl["kg"]
                    gt = st[g]["gt"]
                    aview = gt[:, :, 128:132].bitcast(fp16)  # [P, kg, 4]
                    zt = psm.tile([P, kg, 4], fp16, tag="zt")
                    nc.vector.tensor_tensor(out=zt[:], in0=aview,
                                            in1=st[g]["adt"][:], op=OP.add)
                    lr = psm.tile([P, kg * 4], fp16, tag="lr")
                    nc.vector.scalar_tensor_tensor(
                        out=lr[:], in0=zt[:].rearrange("p k f -> p (k f)"),
                        scalar=NEG, in1=zt[:].rearrange("p k f -> p (k f)"),
                        op0=OP.mult, op1=OP.max)
                    wb = psm.tile([P, kg, 4], fp16, tag="wb")
                    nc.scalar.activation(
                        out=wb[:].rearrange("p k f -> p (k f)"), in_=lr[:],
                        func=AF.Exp)
                    wbx = pms.tile([P, kg, H, C], fp16, tag="wbx")
                    nc.scalar.activation(
                        out=wbx[:],
                        in_=wb[:, :, :, None].to_broadcast([P, kg, H, C]),
                        func=AF.Copy)
                    st[g]["wb"] = wb
                    st[g]["wbx"] = wbx

                def issue_msg2(g):
                    pl = plan[g]
                    kg = pl["kg"]
                    gt = st[g]["gt"]
                    hview = gt[:, :, 0:128].bitcast(bf16)    # [P, kg, 128]
                    msg = pms.tile([P, kg, 132], fp16, tag="msg")
                    nc.vector.tensor_tensor(
                        out=msg[:, :, 0:128],
                        in0=hview,
                        in1=st[g]["wbx"][:].rearrange("p k h c -> p (k h c)")
                            .rearrange("p (k f) -> p k f", k=kg),
                        op=OP.mult)
                    nc.scalar.activation(out=msg[:, :, 128:132],
                                         in_=st[g]["wb"][:], func=AF.Copy)
                    st[g]["msg"] = msg

                def issue_scatter(g):
                    pl = plan[g]
                    msg, s01g = st[g]["msg"], st[g]["s01g"]
                    acc = psb.tile([P, G, 132], f32, tag="acc", space="PSUM")
                    for ti, k, first, last in pl["scatter"]:
                        nc.tensor.matmul(out=acc[:, ti, :], lhsT=s01g[:, k, :],
                                         rhs=msg[:, k, 0:132],
                                         start=first, stop=last)
                    st[g]["acc"] = acc

                def issue_epilogue(g):
                    pl = plan[g]
                    tiles = pl["tiles"]
                    ng = len(tiles)
                    acc = st[g]["acc"]
                    den = psm.tile([P, G, 4], f32, tag="den")
                    nc.vector.reciprocal(out=den[:, 0:ng, :],
                                         in_=acc[:, 0:ng, 128:132])
                    tmp = psm.tile([P, G, P], f32, tag="tmp")
                    nc.vector.tensor_tensor(
                        out=tmp[:, 0:ng, :].rearrange("p g (h c) -> p g h c", h=H),
                        in0=acc[:, 0:ng, 0:128].rearrange("p g (h c) -> p g h c", h=H),
                        in1=den[:, 0:ng, :, None].to_broadcast([P, ng, H, C]),
                        op=OP.mult)
                    hsum = psm.tile([P, G, C], f32, tag="hsum")
                    nc.vector.tensor_reduce(
                        out=hsum[:, 0:ng, :],
                        in_=tmp[:, 0:ng, :].rearrange("p g (h c) -> p g c h", h=H),
                        axis=mybir.AxisListType.X, op=OP.add)
                    badd = psm.tile([P, G, C], f32, tag="badd")
                    nc.vector.scalar_tensor_tensor(
                        out=badd[:, 0:ng, :], in0=hsum[:, 0:ng, :],
                        scalar=1.0 / H,
                        in1=bias_sb[:, None, :].to_broadcast([P, ng, C]),
                        op0=OP.mult, op1=OP.add)
                    for ti, t in enumerate(tiles):
                        nc.vector.tensor_scalar_max(
                            out=outall_sb[:, t, :], in0=badd[:, ti, :],
                            scalar1=0.0)
                    del st[g]

                issue_streams(0)
                for i in range(NG + 2):
                    if 1 <= i <= NG:
                        issue_z(i - 1)
                    if i + 1 < NG:
                        issue_streams(i + 1)
                    if i < NG:
                        issue_gather(i)
                        issue_bitexp(i)
                        issue_adt(i)
                    if 1 <= i <= NG:
                        issue_msg2(i - 1)
                        issue_scatter(i - 1)
                    if i >= 2:
                        issue_epilogue(i - 2)
                nc.sync.dma_start(out=out_v[:, :, :], in_=outall_sb[:])
    nc.compile()
    return nc


def prepare(x, edge_index, W, att_src, att_dst, bias):
    x = np.asarray(x, np.float32)
    W = np.asarray(W, np.float32)
    att_src = np.asarray(att_src, np.float32)
    att_dst = np.asarray(att_dst, np.float32)
    bias = np.asarray(bias, np.float32)

    wa = np.zeros((P, 136), np.float32)
    wa[:, :128] = W
    for hh in range(H):
        wa[:, 128 + hh] = W[:, hh * C:(hh + 1) * C] @ att_src[hh]
        # one-hot entries are 2.0 (bit shifted to fp16 exponent); halve a_dst
        wa[:, 132 + hh] = 0.5 * (W[:, hh * C:(hh + 1) * C] @ att_dst[hh])
    wa_bf = wa.astype(BF16)

    x_pad = np.zeros((NPAD, P), np.float32)
    x_pad[:N] = x
    x_bf = x_pad.astype(BF16)

    K, groups, idx_maps, bm_maps = _prep_edges(np.asarray(edge_index))
    plan, TOTC = _plan(K, groups)
    bmi_maps = _interleave_bitmaps(plan, TOTC, bm_maps)
    nc = _build_program(K, groups)

    bias_rep = np.tile(bias[None, :], (P, 1)).astype(np.float32)

    in_maps = []
    for c in range(NCORE):
        xc = np.roll(x_bf, -c * B, axis=0)
        in_maps.append({
            "xbf": np.ascontiguousarray(xc),
            "waug": wa_bf,
            "idx_all": idx_maps[c],
            "bm_all": bmi_maps[c],
            "bias_in": bias_rep,
        })
    return nc, in_maps


def kernel(x, edge_index, W, att_src, att_dst, bias):
    nc, in_maps = prepare(x, edge_index, W, att_src, att_dst, bias)
    res = run_bass_kernel_spmd(nc, in_maps, list(range(NCORE)))
    out = np.empty((NPAD, C), np.float32)
    for c in range(NCORE):
        out[c * B:(c + 1) * B] = res.results[c]["out"]
    return out[:N]
